# revision 7
# baseline (speedup 1.0000x reference)
"""ComplexLSTM Trainium2 kernel.

Problem: x [2, 64, 128, 1024] (real/imag, B, I, T) -> out [2, 64, 256, 1024].
Four real LSTM applications: lstm_r(x_real), lstm_r(x_imag), lstm_i(x_real),
lstm_i(x_imag); combined as L_r = r(xr) - i(xim), L_i = r(xim) + i(xr).

Sharding: 2 weight-sets x 128 sequences each = 256 independent sequences.
8 cores x 32 sequences (cores 0-3: r-weights, cores 4-7: i-weights).

Device layout (fully transposed state, weights-stationary matmuls):
  gates.T accumulated in PSUM as [128p, 8 blocks, 32 batch] where
  block j = gate rows 128j..128j+127, gate order permuted to [g,g,i,i,f,f,o,o].
  bias enters via an indicator matmul, x-projection via 8 MMs
  (lhsT=WihT tiles, rhs=x_t.T), recurrent part via 16 MMs
  (lhsT=WhhT tiles bf16 FWL, rhs=h.T slices of the history buffer).

I/O is arranged so the host does (almost) no work:
  input  = raw contiguous f32 slice x[part, b0:b0+32]  [32, 128, T]
           (DMA'd with a transposed AP, reordered+cast to bf16 on DVE)
  output = final-layout f32 [32, 2, 128, T] == [32, 256, T] per core
           (DVE writes an f32 copy of h, DMA scatters to the final AP)
  host   = four paired np.subtract/np.add straight into the result array.
"""

import os
import numpy as np
import ml_dtypes
from contextlib import ExitStack

_VAR = os.environ.get("KVAR", "")

import concourse.bass as bass
import concourse.bacc as bacc
import concourse.tile as tile
from concourse import mybir
from concourse.bass_utils import run_bass_kernel_spmd

BF16 = mybir.dt.bfloat16
F32 = mybir.dt.float32
AF = mybir.ActivationFunctionType
OP = mybir.AluOpType

B, I, T_FULL, H = 64, 128, 1024, 256
NB = 32          # batch (sequences) per core
NCORES = 8
CH = 128         # T chunk (steps per input/output DMA)

_cache = {}


def build(T):
    nc = bacc.Bacc("TRN2", target_bir_lowering=False, debug=False)

    ch = max(1, min(CH, T))
    assert T % ch == 0

    xc_d = nc.declare_dram_parameter("xc", [NB, 128, T], F32, isOutput=False)
    whhT_d = nc.declare_dram_parameter("whhT", [128, 2, 8, 128], BF16, isOutput=False)
    wihT_d = nc.declare_dram_parameter("wihT", [128, 8, 128], BF16, isOutput=False)
    biasK_d = nc.declare_dram_parameter("biasK", [8, 128], BF16, isOutput=False)
    ind_d = nc.declare_dram_parameter("ind", [8, 8 * NB], BF16, isOutput=False)
    out_d = nc.declare_dram_parameter("out", [NB, 2, 128, T], F32, isOutput=True)

    with tile.TileContext(nc) as tc, ExitStack() as ctx:
        consts = ctx.enter_context(tc.tile_pool(name="consts", bufs=1))
        xraw = ctx.enter_context(tc.tile_pool(name="xraw", bufs=2))
        xin = ctx.enter_context(tc.tile_pool(name="xin", bufs=2))
        hpool = ctx.enter_context(tc.tile_pool(name="hist", bufs=2))
        lpool = ctx.enter_context(tc.tile_pool(name="lout", bufs=2))
        psum = ctx.enter_context(tc.tile_pool(name="psum", bufs=2, space="PSUM"))
        sml = ctx.enter_context(tc.tile_pool(name="small", bufs=3))
        cpool = ctx.enter_context(tc.tile_pool(name="cpool", bufs=3))

        WHH = consts.tile([128, 2, 8, 128], BF16)
        nc.sync.dma_start(WHH[:], whhT_d[:])
        WIH = consts.tile([128, 8, 128], BF16)
        nc.sync.dma_start(WIH[:], wihT_d[:])
        BIASK = consts.tile([8, 128], BF16)
        nc.sync.dma_start(BIASK[:], biasK_d[:])
        IND = consts.tile([8, 8 * NB], BF16)
        nc.sync.dma_start(IND[:], ind_d[:])

        xcT = xc_d[:].transpose([1, 0, 2])  # [128 i, NB b, T]

        XT = None
        HIST = None
        LOUT = None
        c_prev = None
        h_prev = None  # AP into HIST for h.T(t-1)

        for t in range(T):
            tl = t % ch
            if tl == 0:
                XR = xraw.tile([128, NB, ch], F32, tag="xr")
                nc.sync.dma_start(XR[:], xcT[:, :, t:t + ch])
                XT = xin.tile([128, ch, NB], BF16, tag="xt")
                nc.vector.tensor_copy(XT[:], XR[:].transpose([0, 2, 1]))
                HIST = hpool.tile([128, 2, NB, ch], BF16, tag="hist")
                LOUT = lpool.tile([128, 2, NB, ch], F32, tag="lout")

            g_ps = psum.tile([128, 8, NB], F32, tag="gates")
            # bias (clears PSUM), then x-projection, then recurrent part
            nc.tensor.matmul(g_ps[:], BIASK[:], IND[:], start=True, stop=False)
            for m in range(8):
                nc.tensor.matmul(
                    g_ps[:, m, :], WIH[:, m, :], XT[:, tl, :],
                    start=False, stop=False,
                )
            if t > 0:
                for m in range(8):
                    for k in range(2):
                        nc.tensor.matmul(
                            g_ps[:, m, :], WHH[:, k, m, :], h_prev[:, k, :],
                            start=False, stop=(k == 1),
                        )

            # activations: blocks [0:2]=g (tanh), [2:8]=i,f,o (sigmoid)
            sg = sml.tile([128, 6, NB], F32, tag="sg")
            nc.scalar.activation(sg[:], g_ps[:, 2:8, :], AF.Sigmoid)
            gt = sml.tile([128, 2, NB], F32, tag="gt")
            nc.scalar.activation(gt[:], g_ps[:, 0:2, :], AF.Tanh)

            v = sml.tile([128, 2, NB], F32, tag="v")
            nc.vector.tensor_tensor(v[:], sg[:, 0:2, :], gt[:], OP.mult)
            c_new = cpool.tile([128, 2, NB], F32, tag="c")
            if t > 0:
                u = sml.tile([128, 2, NB], F32, tag="u")
                nc.vector.tensor_tensor(u[:], sg[:, 2:4, :], c_prev[:], OP.mult)
                nc.vector.tensor_tensor(c_new[:], u[:], v[:], OP.add)
            else:
                nc.vector.tensor_copy(c_new[:], v[:])
            tch = sml.tile([128, 2, NB], F32, tag="tch")
            nc.scalar.activation(tch[:], c_new[:], AF.Tanh)
            h_slot = HIST[:, :, :, tl]
            nc.vector.tensor_tensor(h_slot, sg[:, 4:6, :], tch[:], OP.mult)
            # f32 copy of h into the output staging tile (off critical path)
            if "nolout" not in _VAR:
                if "vlout" in _VAR:
                    nc.vector.tensor_copy(LOUT[:, :, :, tl], HIST[:, :, :, tl])
                else:
                    nc.scalar.copy(LOUT[:, :, :, tl], HIST[:, :, :, tl])

            c_prev = c_new
            h_prev = HIST[:, :, :, tl]

            if tl == ch - 1:
                t0 = t - (ch - 1)
                for hc in range(2):
                    nc.sync.dma_start(
                        out_d[:, hc, :, t0:t0 + ch].transpose([1, 0, 2]),
                        LOUT[:, hc, :, :])
    nc.compile()
    return nc


def _get_nc(T):
    if T not in _cache:
        _cache[T] = build(T)
    return _cache[T]


def _prep_weights(Wih, Whh, bih, bhh):
    """Per weight-set host prep of the (small) weight tensors."""
    # gate permutation torch [i,f,g,o] -> [g,i,f,o]
    perm = np.concatenate([np.arange(512, 768), np.arange(0, 256),
                           np.arange(256, 512), np.arange(768, 1024)])
    Wihp = np.asarray(Wih)[perm]          # [1024, 128]
    Whhp = np.asarray(Whh)[perm]          # [1024, 256]
    biasp = (np.asarray(bih) + np.asarray(bhh))[perm]  # [1024]

    whhT = Whhp.reshape(8, 128, 2, 128).transpose(3, 2, 0, 1)  # [p,k,m,j]
    wihT = Wihp.reshape(8, 128, 128).transpose(2, 0, 1)        # [p,m,j]
    biasK = biasp.reshape(8, 128)
    whhT = np.ascontiguousarray(whhT).astype(ml_dtypes.bfloat16)
    wihT = np.ascontiguousarray(wihT).astype(ml_dtypes.bfloat16)
    biasK = biasK.astype(ml_dtypes.bfloat16)
    return whhT, wihT, biasK


def _run(x, Wih_r, Whh_r, bih_r, bhh_r, Wih_i, Whh_i, bih_i, bhh_i, T,
         trace=False, tmpdir=None):
    nc = _get_nc(T)
    ind = np.kron(np.eye(8), np.ones((1, NB))).astype(ml_dtypes.bfloat16)

    w_r = _prep_weights(Wih_r, Whh_r, bih_r, bhh_r)
    w_i = _prep_weights(Wih_i, Whh_i, bih_i, bhh_i)

    x = np.asarray(x)
    in_maps = []
    for core in range(NCORES):
        ws = core // 4
        g = core % 4
        whhT, wihT, biasK = w_r if ws == 0 else w_i
        part, b0 = (0, 32 * g) if g < 2 else (1, 32 * (g - 2))
        in_maps.append({
            "xc": x[part, b0:b0 + 32],
            "whhT": whhT, "wihT": wihT, "biasK": biasK, "ind": ind,
        })
    res = run_bass_kernel_spmd(nc, in_maps, core_ids=list(range(NCORES)),
                               trace=trace, tmpdir=tmpdir)
    results = res.results

    Hc = [results[c]["out"].reshape(NB, 2 * 128, T) for c in range(NCORES)]
    out = np.empty((2, B, 2 * 128, T), np.float32)
    np.subtract(Hc[0], Hc[6], out=out[0, 0:32])
    np.subtract(Hc[1], Hc[7], out=out[0, 32:64])
    np.add(Hc[2], Hc[4], out=out[1, 0:32])
    np.add(Hc[3], Hc[5], out=out[1, 32:64])
    return out, res


def kernel(x, Wih_r, Whh_r, bih_r, bhh_r, Wih_i, Whh_i, bih_i, bhh_i):
    out, _ = _run(x, Wih_r, Whh_r, bih_r, bhh_r,
                  Wih_i, Whh_i, bih_i, bhh_i, T_FULL)
    return out


# revision 11
# speedup vs baseline: 1.6643x; 1.6643x over previous
"""ComplexLSTM Trainium2 kernel.

Problem: x [2, 64, 128, 1024] (real/imag, B, I, T) -> out [2, 64, 256, 1024].
Four real LSTM applications: lstm_r(x_real), lstm_r(x_imag), lstm_i(x_real),
lstm_i(x_imag); combined as L_r = r(xr) - i(xim), L_i = r(xim) + i(xr).

Sharding: 2 weight-sets x 128 sequences each = 256 independent sequences.
8 cores x 32 sequences (cores 0-3: r-weights, cores 4-7: i-weights).

Device layout (fully transposed state, weights-stationary matmuls):
  gates.T accumulated in PSUM as [128p, 8 blocks, 32 batch] where
  block j = gate rows 128j..128j+127, gate order permuted to [g,g,i,i,f,f,o,o].
  bias enters via an indicator matmul, x-projection via 8 MMs
  (lhsT=WihT tiles, rhs=x_t.T), recurrent part via 16 MMs
  (lhsT=WhhT tiles bf16 FWL, rhs=h.T slices of the history buffer).

I/O is arranged so the host does (almost) no work:
  input  = raw contiguous f32 slice x[part, b0:b0+32]  [32, 128, T]
           (DMA'd with a transposed AP, reordered+cast to bf16 on DVE)
  output = final-layout f32 [32, 2, 128, T] == [32, 256, T] per core
           (DVE writes an f32 copy of h, DMA scatters to the final AP)
  host   = four paired np.subtract/np.add straight into the result array.
"""

import os
import numpy as np
import ml_dtypes
from contextlib import ExitStack

_VAR = os.environ.get("KVAR", "")

import concourse.bass as bass
import concourse.bacc as bacc
import concourse.tile as tile
from concourse import mybir
from concourse.bass_utils import run_bass_kernel_spmd

BF16 = mybir.dt.bfloat16
F32 = mybir.dt.float32
AF = mybir.ActivationFunctionType
OP = mybir.AluOpType

B, I, T_FULL, H = 64, 128, 1024, 256
NB = 32          # batch (sequences) per core
NCORES = 8
CH = 128         # T chunk (steps per input/output DMA)

_cache = {}


def build(T):
    nc = bacc.Bacc("TRN2", target_bir_lowering=False, debug=False)

    ch = max(1, min(CH, T))
    assert T % ch == 0

    xc_d = nc.declare_dram_parameter("xc", [NB, 128, T], BF16, isOutput=False)
    whhT_d = nc.declare_dram_parameter("whhT", [128, 2, 8, 128], BF16, isOutput=False)
    wihT_d = nc.declare_dram_parameter("wihT", [128, 8, 128], BF16, isOutput=False)
    biasK_d = nc.declare_dram_parameter("biasK", [8, 128], BF16, isOutput=False)
    ind_d = nc.declare_dram_parameter("ind", [8, 8 * NB], BF16, isOutput=False)
    out_d = nc.declare_dram_parameter("out", [NB, 2, 128, T], BF16, isOutput=True)

    with tile.TileContext(nc) as tc, ExitStack() as ctx:
        consts = ctx.enter_context(tc.tile_pool(name="consts", bufs=1))
        xraw = ctx.enter_context(tc.tile_pool(name="xraw", bufs=2))
        xin = ctx.enter_context(tc.tile_pool(name="xin", bufs=2))
        hpool = ctx.enter_context(tc.tile_pool(name="hist", bufs=2))
        lpool = ctx.enter_context(tc.tile_pool(name="lout", bufs=2))
        psum = ctx.enter_context(tc.tile_pool(name="psum", bufs=2, space="PSUM"))
        sml = ctx.enter_context(tc.tile_pool(name="small", bufs=3))
        cpool = ctx.enter_context(tc.tile_pool(name="cpool", bufs=3))

        WHH = consts.tile([128, 2, 8, 128], BF16)
        nc.sync.dma_start(WHH[:], whhT_d[:])
        WIH = consts.tile([128, 8, 128], BF16)
        nc.sync.dma_start(WIH[:], wihT_d[:])
        BIASK = consts.tile([8, 128], BF16)
        nc.sync.dma_start(BIASK[:], biasK_d[:])
        IND = consts.tile([8, 8 * NB], BF16)
        nc.sync.dma_start(IND[:], ind_d[:])

        xcT = xc_d[:].transpose([1, 0, 2])  # [128 i, NB b, T]

        XT = None
        HIST = None
        LOUT = None
        c_prev = None
        h_prev = None  # AP into HIST for h.T(t-1)

        for t in range(T):
            tl = t % ch
            if tl == 0:
                XR = xraw.tile([128, NB, ch], BF16, tag="xr")
                nc.sync.dma_start(XR[:], xcT[:, :, t:t + ch])
                XT = xin.tile([128, ch, NB], BF16, tag="xt")
                nc.vector.tensor_copy(XT[:], XR[:].transpose([0, 2, 1]))
                HIST = hpool.tile([128, 2, NB, ch], BF16, tag="hist")

            g_ps = psum.tile([128, 8, NB], F32, tag="gates")
            # bias (clears PSUM), then x-projection, then recurrent part
            nc.tensor.matmul(g_ps[:], BIASK[:], IND[:], start=True, stop=False)
            for m in range(8):
                nc.tensor.matmul(
                    g_ps[:, m, :], WIH[:, m, :], XT[:, tl, :],
                    start=False, stop=False,
                )
            if t > 0:
                for m in range(8):
                    for k in range(2):
                        nc.tensor.matmul(
                            g_ps[:, m, :], WHH[:, k, m, :], h_prev[:, k, :],
                            start=False, stop=(k == 1),
                        )

            # activations: blocks [0:2]=g (tanh), [2:8]=i,f,o (sigmoid)
            sg = sml.tile([128, 6, NB], F32, tag="sg")
            nc.scalar.activation(sg[:], g_ps[:, 2:8, :], AF.Sigmoid)
            gt = sml.tile([128, 2, NB], F32, tag="gt")
            nc.scalar.activation(gt[:], g_ps[:, 0:2, :], AF.Tanh)

            v = sml.tile([128, 2, NB], F32, tag="v")
            nc.vector.tensor_tensor(v[:], sg[:, 0:2, :], gt[:], OP.mult)
            c_new = cpool.tile([128, 2, NB], F32, tag="c")
            if t > 0:
                u = sml.tile([128, 2, NB], F32, tag="u")
                nc.vector.tensor_tensor(u[:], sg[:, 2:4, :], c_prev[:], OP.mult)
                nc.vector.tensor_tensor(c_new[:], u[:], v[:], OP.add)
            else:
                nc.vector.tensor_copy(c_new[:], v[:])
            tch = sml.tile([128, 2, NB], F32, tag="tch")
            nc.scalar.activation(tch[:], c_new[:], AF.Tanh)
            h_slot = HIST[:, :, :, tl]
            nc.vector.tensor_tensor(h_slot, sg[:, 4:6, :], tch[:], OP.mult)

            c_prev = c_new
            h_prev = HIST[:, :, :, tl]

            if tl == ch - 1:
                t0 = t - (ch - 1)
                for hc in range(2):
                    nc.sync.dma_start(
                        out_d[:, hc, :, t0:t0 + ch].transpose([1, 0, 2]),
                        HIST[:, hc, :, :])
    nc.compile()
    return nc


def _get_nc(T):
    if T not in _cache:
        _cache[T] = build(T)
    return _cache[T]


def _prep_weights(Wih, Whh, bih, bhh):
    """Per weight-set host prep of the (small) weight tensors."""
    # gate permutation torch [i,f,g,o] -> [g,i,f,o]
    perm = np.concatenate([np.arange(512, 768), np.arange(0, 256),
                           np.arange(256, 512), np.arange(768, 1024)])
    Wihp = np.asarray(Wih)[perm]          # [1024, 128]
    Whhp = np.asarray(Whh)[perm]          # [1024, 256]
    biasp = (np.asarray(bih) + np.asarray(bhh))[perm]  # [1024]

    whhT = Whhp.reshape(8, 128, 2, 128).transpose(3, 2, 0, 1)  # [p,k,m,j]
    wihT = Wihp.reshape(8, 128, 128).transpose(2, 0, 1)        # [p,m,j]
    biasK = biasp.reshape(8, 128)
    whhT = np.ascontiguousarray(whhT).astype(ml_dtypes.bfloat16)
    wihT = np.ascontiguousarray(wihT).astype(ml_dtypes.bfloat16)
    biasK = biasK.astype(ml_dtypes.bfloat16)
    return whhT, wihT, biasK


def _bf16_rne(a):
    """float32 -> bfloat16 with round-to-nearest-even, vectorized."""
    u = np.asarray(a, np.float32).view(np.uint32)
    r = ((u + 0x7FFF + ((u >> 16) & 1)) >> 16).astype(np.uint16)
    return r.view(ml_dtypes.bfloat16)


def _f32_from_bf16(a):
    """bfloat16 -> float32, vectorized (exact)."""
    return (a.view(np.uint16).astype(np.uint32) << 16).view(np.float32)


def _run(x, Wih_r, Whh_r, bih_r, bhh_r, Wih_i, Whh_i, bih_i, bhh_i, T,
         trace=False, tmpdir=None):
    nc = _get_nc(T)
    ind = np.kron(np.eye(8), np.ones((1, NB))).astype(ml_dtypes.bfloat16)

    w_r = _prep_weights(Wih_r, Whh_r, bih_r, bhh_r)
    w_i = _prep_weights(Wih_i, Whh_i, bih_i, bhh_i)

    xb = _bf16_rne(x)  # [2, B, 128, T] bf16, one pass
    in_maps = []
    for core in range(NCORES):
        ws = core // 4
        g = core % 4
        whhT, wihT, biasK = w_r if ws == 0 else w_i
        part, b0 = (0, 32 * g) if g < 2 else (1, 32 * (g - 2))
        in_maps.append({
            "xc": xb[part, b0:b0 + 32],
            "whhT": whhT, "wihT": wihT, "biasK": biasK, "ind": ind,
        })
    res = run_bass_kernel_spmd(nc, in_maps, core_ids=list(range(NCORES)),
                               trace=trace, tmpdir=tmpdir)
    results = res.results

    Hc = [_f32_from_bf16(results[c]["out"]).reshape(NB, 2 * 128, T)
          for c in range(NCORES)]
    out = np.empty((2, B, 2 * 128, T), np.float32)
    np.subtract(Hc[0], Hc[6], out=out[0, 0:32])
    np.subtract(Hc[1], Hc[7], out=out[0, 32:64])
    np.add(Hc[2], Hc[4], out=out[1, 0:32])
    np.add(Hc[3], Hc[5], out=out[1, 32:64])
    return out, res


def kernel(x, Wih_r, Whh_r, bih_r, bhh_r, Wih_i, Whh_i, bih_i, bhh_i):
    out, _ = _run(x, Wih_r, Whh_r, bih_r, bhh_r,
                  Wih_i, Whh_i, bih_i, bhh_i, T_FULL)
    return out
